# revision 1
# baseline (speedup 1.0000x reference)
"""Fused DFFN kernel for Trainium2, 8 NeuronCores.

Pipeline (per core, SPMD over 8 cores = 2 batches x 4 H-quarters):
  host: x patches are pre-transformed by the channel-independent real-rfft2
  basis G80 [80,64] (fp32 BLAS GEMM), slabs carry [c, strip, wb, 80] fp16 with
  one patch-strip halo each side (zero-padded at image edges)
  -> proj_in + spectral analysis   : PE matmul with xG as stationary operand
                                     accumulating [80freq, 384ch] per patch
  -> per-channel filter            : diagonal in the 80-dim redundant basis
                                     (one DVE multiply), inverse via H80 [64,80]
                                     matmuls with Zf as stationary operand
  -> depthwise 3x3 conv            : DVE scalar_tensor_tensor per-partition MACs
  -> gated exact GELU              : ACT Gelu + DVE multiply
  -> proj_out (1x1 conv)           : PE matmul
  -> y slab [384, 64, 256] fp32

Everything is hardcoded for B=2, DIM=HID=384, H=W=256, P=8.
"""
import numpy as np

B, DIM, H, W = 2, 384, 256, 256
HID = DIM
P = 8
NCORES = 8
RSTRIP = 8                  # rows per strip (= patch size)
NSTRIP = 10                 # strips per slab (8 own + 2 halo)
SLAB_R = NSTRIP * RSTRIP    # 80
OWN_R = 64
WPAD = 260                  # padded row length for u tiles (guard cols 0,1 and 258,259)
COL0 = 2                    # first data column in u tiles

_compiled = None


# ---------------------------------------------------------------------------
# Host-side math: spectral basis matrices
# ---------------------------------------------------------------------------
def _build_spectral():
    G = np.zeros((80, 64), np.float64)
    for s in range(64):
        e = np.zeros((8, 8)); e.flat[s] = 1.0
        F = np.fft.rfft2(e)
        G[0::2, s] = F.real.flatten()
        G[1::2, s] = F.imag.flatten()
    Hm = np.zeros((64, 80), np.float64)
    for j in range(80):
        z = np.zeros(80); z[j] = 1.0
        F = (z[0::2] + 1j * z[1::2]).reshape(8, 5)
        Hm[:, j] = np.fft.irfft2(F, s=(8, 8)).flatten()
    return G.astype(np.float32), Hm.astype(np.float32)


# channel permutation: chunk0 = x1[0:128], chunk1 = x2[192:320],
# chunk2 = [x1[128:192] | x2[320:384]]
_PERM = np.concatenate([np.arange(0, 128), np.arange(192, 320),
                        np.arange(128, 192), np.arange(320, 384)])

# depthwise-conv taps computed on the tensor engine (diagonal-weight matmuls),
# per channel chunk; the rest run on the vector engine
PE_TAPS = [
    [],
    [(-1, -1), (-1, 0), (-1, 1), (0, -1), (1, -1), (1, 0), (1, 1)],
    [(dy, dx) for dy in (-1, 0, 1) for dx in (-1, 0, 1)],
]
DG_LIST = [(ch, dy, dx) for ch in range(3) for (dy, dx) in PE_TAPS[ch]]
DG_IDX = {t: i for i, t in enumerate(DG_LIST)}
NDG = len(DG_LIST)


# ---------------------------------------------------------------------------
# Bass program
# ---------------------------------------------------------------------------
def _build_program():
    from contextlib import ExitStack
    import concourse.bacc as bacc
    import concourse.mybir as mybir
    import concourse.tile as tile

    f16 = mybir.dt.float16
    f32 = mybir.dt.float32
    MULT = mybir.AluOpType.mult
    ADD = mybir.AluOpType.add
    GELU = mybir.ActivationFunctionType.Gelu

    nc = bacc.Bacc("TRN2", target_bir_lowering=False, debug=False)

    # G-transformed x slab: [c, strip, wb, 80] (host applies G80 per patch)
    xs = nc.dram_tensor("xs", (HID, NSTRIP, W // P, 80), f16, kind="ExternalInput")
    wI = nc.dram_tensor("wI", (DIM, HID), f16, kind="ExternalInput")       # [c_in, o']
    h80r = nc.dram_tensor("h80r", (80, 64), f16, kind="ExternalInput")     # H80^T
    d80 = nc.dram_tensor("d80", (80, HID), f16, kind="ExternalInput")
    taps = nc.dram_tensor("taps", (128, 3, 9), f32, kind="ExternalInput")
    wO0 = nc.dram_tensor("wO0", (128, DIM), f16, kind="ExternalInput")     # g[0:128] rows
    wO1 = nc.dram_tensor("wO1", (64, DIM), f16, kind="ExternalInput")      # g[128:192] rows
    dg = nc.dram_tensor("dg", (128, NDG * 128), f16, kind="ExternalInput")
    y = nc.dram_tensor("y", (DIM, OWN_R, W), f32, kind="ExternalOutput")

    NG = W // (2 * P)          # 16 two-patch groups per strip
    NPATCH = W // P            # 32 patches per strip

    with tile.TileContext(nc) as tc, ExitStack() as ctx:
        wpool = ctx.enter_context(tc.tile_pool(name="weights", bufs=1))
        xpool = ctx.enter_context(tc.tile_pool(name="x", bufs=2))
        spool = ctx.enter_context(tc.tile_pool(name="spec", bufs=4))
        upool = ctx.enter_context(tc.tile_pool(name="u", bufs=4))
        vpool = ctx.enter_context(tc.tile_pool(name="v", bufs=2))
        gpool = ctx.enter_context(tc.tile_pool(name="g", bufs=2))
        opool = ctx.enter_context(tc.tile_pool(name="o", bufs=1))
        pA = ctx.enter_context(tc.tile_pool(name="pA", bufs=2, space="PSUM"))
        pZ = ctx.enter_context(tc.tile_pool(name="pZ", bufs=2, space="PSUM"))
        pH = ctx.enter_context(tc.tile_pool(name="pH", bufs=2, space="PSUM"))
        pV = ctx.enter_context(tc.tile_pool(name="pV", bufs=1, space="PSUM"))

        # ---- preload weights ----
        wI_sb = wpool.tile([128, 3, HID], f16)
        for kc in range(3):
            nc.sync.dma_start(wI_sb[:, kc, :], wI[kc * 128:(kc + 1) * 128, :])
        h80r_sb = wpool.tile([80, 64], f16)
        nc.sync.dma_start(h80r_sb[:], h80r[:])
        d80_sb = wpool.tile([80, HID], f16)
        nc.sync.dma_start(d80_sb[:], d80[:])
        taps_sb = wpool.tile([128, 3, 9], f32)
        nc.sync.dma_start(taps_sb[:], taps[:])
        wO0_sb = wpool.tile([128, DIM], f16)
        nc.sync.dma_start(wO0_sb[:], wO0[:])
        wO1_sb = wpool.tile([64, DIM], f16)
        nc.sync.dma_start(wO1_sb[:], wO1[:])
        dg_sb = wpool.tile([128, NDG * 128], f16)
        nc.sync.dma_start(dg_sb[:], dg[:])

        u_tiles = [None] * NSTRIP
        us_tiles = [None] * NSTRIP

        def spectral(k):
            """proj_in + spectral filter for strip k -> u_tiles[k] (raster fp16,
            guard cols zeroed)."""
            x_sb = xpool.tile([128, 3, (W // P) * 80], f16, tag="x")
            nc.sync.dma_start(
                x_sb.rearrange("c kc (wb f) -> c kc wb f", wb=W // P),
                xs[:, k].rearrange("(kc c) wb f -> c kc wb f", kc=3))

            u_sb = upool.tile([128, 3, RSTRIP, WPAD], f16, tag="u")
            u_tiles[k] = u_sb
            # zero the guard columns
            nc.vector.memset(u_sb[:, :, :, 0:COL0], 0.0)
            nc.vector.memset(u_sb[:, :, :, COL0 + W:WPAD], 0.0)

            for g in range(NG):
                zf = spool.tile([80, 2, HID], f16, tag="zf")
                for p in range(2):
                    pt = 2 * g + p
                    zp = pZ.tile([80, 512], f32, tag="z", name=f"zp_{k}_{g}_{p}")
                    for kc in range(3):
                        nc.tensor.matmul(zp[:, 0:HID],
                                         x_sb[:, kc, pt * 80:(pt + 1) * 80],
                                         wI_sb[:, kc, :],
                                         start=(kc == 0), stop=(kc == 2))
                    nc.vector.tensor_mul(zf[:, p, :], zp[:, 0:HID], d80_sb[:])

                # halo strips only feed one u row into the dwconv: compute
                # just that row of the inverse transform
                r0, r1 = (7, 8) if k == 0 else (0, 1) if k == NSTRIP - 1 else (0, P)
                nr = r1 - r0
                ph = pH.tile([128, 3, 2 * 64], f32, tag="ph", name=f"ph_{k}_{g}")
                for p in range(2):
                    for ch in range(3):
                        nc.tensor.matmul(
                            ph[:, ch, p * 64:p * 64 + nr * P],
                            zf[:, p, ch * 128:(ch + 1) * 128],
                            h80r_sb[:, r0 * P:r1 * P])
                wb0 = g * 2
                for ch in range(3):
                    dst = u_sb[:, ch, r0:r1, COL0 + wb0 * P:COL0 + (wb0 + 2) * P] \
                        .rearrange("c p1 (pt p2) -> c pt p1 p2", pt=2)
                    src = ph[:, ch, :].rearrange("c (pt s) -> c pt s", pt=2) \
                        [:, :, 0:nr * P] \
                        .rearrange("c pt (p1 p2) -> c pt p1 p2", p1=nr)
                    nc.scalar.copy(dst, src)

        def dw_sources(k, ch, dy, dx):
            """(out_rows_slice, src_ap) pairs for one tap over strip k's 8 rows."""
            um, u0, up = u_tiles[k - 1], u_tiles[k], u_tiles[k + 1]
            off = COL0 + dx
            if dy == 0:
                return [((0, 8), u0[:, ch, :, off:off + W])]
            if dy == -1:
                return [((1, 8), u0[:, ch, 0:7, off:off + W]),
                        ((0, 1), um[:, ch, 7:8, off:off + W])]
            return [((0, 7), u0[:, ch, 1:8, off:off + W]),
                    ((7, 8), up[:, ch, 0:1, off:off + W])]

        def dwconv_gate_out(k):
            """dwconv + gelu gate + proj_out + store for own strip k (1..8).

            Taps in PE_TAPS[ch] run as diagonal-weight matmuls accumulating in
            PSUM (half-strip tiles); the rest run as DVE per-partition MACs.
            For mixed chunks the first DVE tap consumes the PSUM accumulator.
            """
            v_sb = vpool.tile([128, 3, RSTRIP, W], f16, tag="v")
            for ch in range(3):
                tp = lambda t: taps_sb[:, ch, t:t + 1]
                tnum = lambda dy, dx: (dy + 1) * 3 + dx + 1
                pe_taps = PE_TAPS[ch]
                dve_taps = [(dy, dx) for dy in (-1, 0, 1) for dx in (-1, 0, 1)
                            if (dy, dx) not in pe_taps]
                if pe_taps and dve_taps:
                    # ensure the psum-consuming first DVE op is the split-free
                    # center tap
                    assert (0, 0) in dve_taps
                    dve_taps.remove((0, 0))
                    dve_taps.insert(0, (0, 0))
                pv_halves = []
                for h in range(2):  # half-strips of 4 rows
                    if not pe_taps:
                        break
                    pv = pV.tile([128, 4 * W], f32, tag="pv",
                                 name=f"pv_{k}_{ch}_{h}")
                    pv_halves.append(pv)
                    nti = len(pe_taps)
                    for ti, (dy, dx) in enumerate(pe_taps):
                        for pi, ((r0, r1), src) in enumerate(dw_sources(k, ch, dy, dx)):
                            for q in (2 * h, 2 * h + 1):  # quarters (2 rows)
                                lo, hi = max(r0, 2 * q), min(r1, 2 * q + 2)
                                if lo >= hi:
                                    continue
                                dgi = DG_IDX[(ch, dy, dx)]
                                # start=True zeroes the whole psum bank region:
                                # only the chronologically first write per
                                # quarter may carry it (later first-touch
                                # writes are handled by has_written bits)
                                nc.tensor.matmul(
                                    pv[:, (lo - 4 * h) * W:(hi - 4 * h) * W],
                                    dg_sb[:, dgi * 128:(dgi + 1) * 128],
                                    src[:, lo - r0:lo - r0 + (hi - lo)],
                                    start=(ti == 0 and pi == 0),
                                    stop=(ti == nti - 1),
                                    skip_group_check=True)
                if pe_taps and not dve_taps:
                    for h in range(2):
                        nc.scalar.copy(
                            v_sb[:, ch, 4 * h:4 * h + 4],
                            pv_halves[h].rearrange("c (r w) -> c r w", r=4))
                for i, (dy, dx) in enumerate(dve_taps):
                    if i == 0 and pe_taps:
                        # center tap, consuming the PE partial sums per half
                        for h in range(2):
                            u0 = u_tiles[k]
                            nc.vector.scalar_tensor_tensor(
                                v_sb[:, ch, 4 * h:4 * h + 4],
                                u0[:, ch, 4 * h:4 * h + 4, COL0:COL0 + W],
                                tp(4),
                                pv_halves[h].rearrange("c (r w) -> c r w", r=4),
                                MULT, ADD)
                        continue
                    for (r0, r1), src in dw_sources(k, ch, dy, dx):
                        dst = v_sb[:, ch, r0:r1]
                        if i == 0:
                            nc.vector.tensor_scalar_mul(dst, src, tp(tnum(dy, dx)))
                        else:
                            nc.vector.scalar_tensor_tensor(
                                dst, src, tp(tnum(dy, dx)), dst, MULT, ADD)

            # gated gelu
            a0 = gpool.tile([128, RSTRIP, W], f16, tag="a0")
            nc.scalar.activation(a0[:], v_sb[:, 0], GELU)
            g0 = gpool.tile([128, RSTRIP, W], f16, tag="g0")
            nc.vector.tensor_mul(g0[:], a0[:], v_sb[:, 1])
            # tail: x1 = v[:, 2][0:64], x2 = v[:, 2][64:128] -> shift x2 to parts 0..63
            x2t = gpool.tile([64, RSTRIP, W], f16, tag="x2t")
            nc.gpsimd.dma_start(x2t[:], v_sb[64:128, 2])
            at = gpool.tile([64, RSTRIP, W], f16, tag="at")
            nc.scalar.activation(at[:], v_sb[0:64, 2], GELU)
            gt = gpool.tile([64, RSTRIP, W], f16, tag="gt")
            nc.vector.tensor_mul(gt[:], at[:], x2t[:])

            # proj_out
            o_sb = opool.tile([128, 3, RSTRIP * W], f32, tag="o")
            g0f = g0.rearrange("c r w -> c (r w)")
            gtf = gt.rearrange("c r w -> c (r w)")
            NT = RSTRIP * W // 512
            for m in range(3):
                for nt in range(NT):
                    pso = pA.tile([128, 512], f32, tag="ps", name=f"pso_{k}_{m}_{nt}")
                    nc.tensor.matmul(pso[:], wO0_sb[:, m * 128:(m + 1) * 128],
                                     g0f[:, nt * 512:(nt + 1) * 512],
                                     start=True, stop=False)
                    nc.tensor.matmul(pso[:], wO1_sb[:, m * 128:(m + 1) * 128],
                                     gtf[:, nt * 512:(nt + 1) * 512],
                                     start=False, stop=True)
                    nc.scalar.copy(o_sb[:, m, nt * 512:(nt + 1) * 512], pso[:])
            nc.gpsimd.dma_start(
                y[:, (k - 1) * RSTRIP:k * RSTRIP, :]
                .rearrange("(m c) r w -> c m r w", m=3),
                o_sb.rearrange("c m (r w) -> c m r w", r=RSTRIP))

        for k in range(NSTRIP):
            spectral(k)
            if k >= 2:
                dwconv_gate_out(k - 1)

    nc.compile()
    return nc


def _prepare_weights(fft_filter, w_in, w_dw, w_out):
    G80, H80 = _build_spectral()
    d80 = np.repeat(fft_filter.reshape(HID, 40), 2, axis=1)  # [hid, 80]
    perm = _PERM
    wI = np.ascontiguousarray(w_in[perm, :].T).astype(np.float16)      # [c_in, o']
    h80r = np.ascontiguousarray(H80.T).astype(np.float16)              # [80, 64]
    d80p = np.ascontiguousarray(d80[perm].T).astype(np.float16)        # [80, o']
    tapsP = w_dw[perm, 0].reshape(3, 128, 9).transpose(1, 0, 2)        # [128, 3, 9]
    tapsP = np.ascontiguousarray(tapsP).astype(np.float32)
    wO0 = np.ascontiguousarray(w_out[:, 0:128].T).astype(np.float16)   # [128, 384]
    wO1 = np.ascontiguousarray(w_out[:, 128:192].T).astype(np.float16) # [64, 384]
    dgm = np.zeros((128, NDG * 128), np.float16)
    for i, (ch, dy, dx) in enumerate(DG_LIST):
        t = (dy + 1) * 3 + dx + 1
        np.fill_diagonal(dgm[:, i * 128:(i + 1) * 128], tapsP[:, ch, t])
    return dict(wI=wI, h80r=h80r, d80=d80p, taps=tapsP,
                wO0=wO0, wO1=wO1, dg=dgm)


def build_in_maps(inputs):
    wd = _prepare_weights(np.asarray(inputs["fft_filter"]), np.asarray(inputs["w_in"]),
                          np.asarray(inputs["w_dw"]), np.asarray(inputs["w_out"]))
    # G-transform the whole image once on the host (channel-independent):
    # [b, c, hb, wb, 64] @ G80.T -> [b, c, hb, wb, 80], fp16 to match the
    # on-device rounding of matmul inputs
    G80, _ = _build_spectral()
    x16 = np.asarray(inputs["x"]).astype(np.float16).astype(np.float32)
    HB = H // P
    xp = x16.reshape(B, HID, HB, P, W // P, P).transpose(0, 1, 2, 4, 3, 5) \
        .reshape(B, HID, HB, W // P, 64)
    xG = (xp.reshape(-1, 64) @ G80.T.astype(np.float32)) \
        .reshape(B, HID, HB, W // P, 80).astype(np.float16)
    in_maps = []
    for core in range(NCORES):
        b, j = divmod(core, 4)
        lo, hi = 8 * j - 1, 8 * j + 9        # patch-strip indices incl halo
        clo, chi = max(lo, 0), min(hi, HB)
        slab = np.zeros((HID, NSTRIP, W // P, 80), np.float16)
        slab[:, clo - lo:clo - lo + (chi - clo)] = xG[b, :, clo:chi]
        in_maps.append({"xs": np.ascontiguousarray(slab), **wd})
    return in_maps


def kernel(x, fft_filter, w_in, w_dw, w_out):
    global _compiled
    import os
    # the axon NTFF profile hook is not shipped in this container; make sure
    # run_bass_kernel_spmd never takes the trace path
    os.environ["BASS_NEVER_TRACE"] = "1"
    from concourse.bass_utils import run_bass_kernel_spmd

    if _compiled is None:
        _compiled = _build_program()
    nc = _compiled

    in_maps = build_in_maps(dict(x=x, fft_filter=fft_filter, w_in=w_in,
                                 w_dw=w_dw, w_out=w_out))

    res = run_bass_kernel_spmd(nc, in_maps, list(range(NCORES)))
    global last_results
    last_results = res
    out = np.empty((B, DIM, H, W), np.float32)
    for core in range(NCORES):
        b, j = divmod(core, 4)
        out[b, :, 64 * j:64 * j + OWN_R, :] = res.results[core]["y"]
    return out



# revision 26
# speedup vs baseline: 1.1963x; 1.1963x over previous
"""Fused DFFN kernel for Trainium2, 8 NeuronCores.

Pipeline (per core, SPMD over 8 cores = 2 batches x 4 H-quarters):
  The per-patch rfft2 -> real diagonal filter -> irfft2 composite is, for
  every channel, a circular convolution with an even-symmetric kernel, so the
  whole filter family shares one orthogonal 64-dim eigenbasis V:
      T_c = V diag(L @ d_c) V^T            (exact, channel-independent V)
  Host pre-transforms x patches by V (fp32 BLAS), slabs carry
  [kc, c, strip, wb, 64] fp16 with one patch-strip halo each side.
  -> proj_in + analysis  : PE matmul, two patches packed per 128-row PSUM tile
  -> per-channel filter  : diagonal in the 64-dim eigenbasis; the PSUM->SBUF
                           eviction IS the filter multiply (DVE/Pool split)
  -> inverse transform   : PE matmuls with zf stationary (base partition 0/64)
  -> depthwise 3x3 conv  : chunk0 fully on PE (diag-weight matmuls) with the
                           GELU fused into the PSUM eviction; chunks 1/2 on
                           DVE 4x tensor_scalar products + TT adds, Pool
                           fused MACs, and gpsimd DMA accumulates
  -> gated exact GELU    : ACT Gelu + DVE multiply
  -> proj_out (1x1 conv) : PE matmul, fp16 output (host upcasts)

Everything is hardcoded for B=2, DIM=HID=384, H=W=256, P=8.
"""
import numpy as np

B, DIM, H, W = 2, 384, 256, 256
HID = DIM
P = 8
NCORES = 8
RSTRIP = 8                  # rows per strip (= patch size)
NSTRIP = 10                 # strips per slab (8 own + 2 halo)
OWN_R = 64
WPAD = 260                  # padded row length for u tiles (guard cols)
COL0 = 2                    # first data column in u tiles
NWB = W // P                # 32 patches per strip
NG = NWB // 2               # 16 two-patch groups per strip

_compiled = None


# ---------------------------------------------------------------------------
# Host-side math: 64-dim shared eigenbasis of the spectral filter family
# ---------------------------------------------------------------------------
def _build_spectral64():
    G = np.zeros((80, 64), np.float64)
    for s in range(64):
        e = np.zeros((8, 8)); e.flat[s] = 1.0
        F = np.fft.rfft2(e)
        G[0::2, s] = F.real.flatten()
        G[1::2, s] = F.imag.flatten()
    Hm = np.zeros((64, 80), np.float64)
    for j in range(80):
        z = np.zeros(80); z[j] = 1.0
        F = (z[0::2] + 1j * z[1::2]).reshape(8, 5)
        Hm[:, j] = np.fft.irfft2(F, s=(8, 8)).flatten()

    def T_of(dflat):
        rep = np.repeat(dflat, 2)
        return Hm @ (rep[:, None] * G)

    rng = np.random.default_rng(12345)
    M = T_of(rng.normal(size=40))
    M = (M + M.T) / 2
    _, V = np.linalg.eigh(M)            # shared orthogonal eigenbasis
    L = np.zeros((64, 40))
    for i in range(40):
        d = np.zeros(40); d[i] = 1.0
        L[:, i] = np.diag(V.T @ T_of(d) @ V)
    # self-check the factorization
    d = rng.normal(size=40)
    T = T_of(d)
    R = V @ ((L @ d)[:, None] * V.T)
    assert np.abs(R - T).max() < 1e-10 * np.abs(T).max()
    return V, L


# channel permutation: chunk0 = x1[0:128], chunk1 = x2[192:320],
# chunk2 = [x1[128:192] | x2[320:384]]
_PERM = np.concatenate([np.arange(0, 128), np.arange(192, 320),
                        np.arange(128, 192), np.arange(320, 384)])

TAPS9 = [(dy, dx) for dy in (-1, 0, 1) for dx in (-1, 0, 1)]
# chunk0 runs fully on the tensor engine (diag-weight matmuls)
PE_TAPS0 = TAPS9
# chunks 1/2: engine per tap; (0,0) must stay "init" (DVE tensor_scalar)
#   init: v = w*u   | dve: tmp = w*u; v += tmp | pool: fused MAC on gpsimd
#   dma:  tmp = w*u (DVE); v += tmp via software-DGE accumulate DMA
CH_PLAN = {
    (0, 0): "init",
    (-1, -1): "dve", (-1, 0): "dve", (-1, 1): "act",
    (0, -1): "dma", (0, 1): "dma",
    (1, -1): "dma", (1, 0): "dma", (1, 1): "dma",
}
# psum->sbuf filter eviction engines, cycled per patch-pair group:
#   a: DVE multiply from PSUM | b: ACT copy + Pool mult | c: ACT copy + DVE mult
ZF_PLAN = ["a", "b", "c", "a", "b", "c", "a", "c", "a", "b"]


# ---------------------------------------------------------------------------
# Bass program
# ---------------------------------------------------------------------------
def _build_program():
    import os
    from contextlib import ExitStack
    import concourse.bacc as bacc
    import concourse.mybir as mybir
    import concourse.tile as tile

    safe = os.environ.get("K_SAFE") == "1"
    no_dma_acc = safe or os.environ.get("K_NO_DMA_ACC") == "1"
    no_pool_zf = safe or os.environ.get("K_NO_POOL_ZF") == "1"
    no_act_tap = safe or os.environ.get("K_NO_ACT_TAP") == "1"
    no_gelu_fuse = safe or os.environ.get("K_NO_GELU_FUSE") == "1"
    no_zf_c = safe or os.environ.get("K_NO_ZF_C") == "1"
    no_dve_pso = safe or os.environ.get("K_NO_DVE_PSO") == "1"
    no_y16 = os.environ.get("K_NO_Y16") == "1"

    f16 = mybir.dt.float16
    f32 = mybir.dt.float32
    MULT = mybir.AluOpType.mult
    ADD = mybir.AluOpType.add
    GELU = mybir.ActivationFunctionType.Gelu

    nc = bacc.Bacc("TRN2", target_bir_lowering=False, debug=False)

    # V-transformed x slab: [kc, c, strip, wb, 64] (host applies V per patch)
    xs = nc.dram_tensor("xs", (3, 128, NSTRIP, NWB, 64), f16,
                        kind="ExternalInput")
    wI = nc.dram_tensor("wI", (DIM, HID), f16, kind="ExternalInput")
    h64 = nc.dram_tensor("h64", (128, 2, 64), f16, kind="ExternalInput")
    d64 = nc.dram_tensor("d64", (128, HID), f16, kind="ExternalInput")
    taps = nc.dram_tensor("taps", (128, 3, 9), f32, kind="ExternalInput")
    wO0 = nc.dram_tensor("wO0", (128, DIM), f16, kind="ExternalInput")
    wO1 = nc.dram_tensor("wO1", (64, DIM), f16, kind="ExternalInput")
    dg = nc.dram_tensor("dg", (128, 9 * 128), f16, kind="ExternalInput")
    y = nc.dram_tensor("y", (DIM, OWN_R, W), f16, kind="ExternalOutput")

    with tile.TileContext(nc) as tc, ExitStack() as ctx:
        wpool = ctx.enter_context(tc.tile_pool(name="weights", bufs=1))
        xpool = ctx.enter_context(tc.tile_pool(name="x", bufs=2))
        spool = ctx.enter_context(tc.tile_pool(name="spec", bufs=4))
        upool = ctx.enter_context(tc.tile_pool(name="u", bufs=4))
        vpool = ctx.enter_context(tc.tile_pool(name="v", bufs=2))
        tpool = ctx.enter_context(tc.tile_pool(name="t", bufs=6))
        gpool = ctx.enter_context(tc.tile_pool(name="g", bufs=2))
        opool = ctx.enter_context(tc.tile_pool(name="o", bufs=2))
        pZ = ctx.enter_context(tc.tile_pool(name="pZ", bufs=2, space="PSUM"))
        pH = ctx.enter_context(tc.tile_pool(name="pH", bufs=2, space="PSUM"))
        pV = ctx.enter_context(tc.tile_pool(name="pV", bufs=2, space="PSUM"))
        pA = ctx.enter_context(tc.tile_pool(name="pA", bufs=2, space="PSUM"))

        # ---- preload weights ----
        wI_sb = wpool.tile([128, 3, HID], f16)
        for kc in range(3):
            nc.sync.dma_start(wI_sb[:, kc, :], wI[kc * 128:(kc + 1) * 128, :])
        h64_sb = wpool.tile([128, 2, 64], f16)
        nc.sync.dma_start(h64_sb[:], h64[:])
        d64_sb = wpool.tile([128, HID], f16)
        nc.sync.dma_start(d64_sb[:], d64[:])
        taps_sb = wpool.tile([128, 3, 9], f32)
        nc.sync.dma_start(taps_sb[:], taps[:])
        wO0_sb = wpool.tile([128, DIM], f16)
        nc.sync.dma_start(wO0_sb[:], wO0[:])
        wO1_sb = wpool.tile([64, DIM], f16)
        nc.sync.dma_start(wO1_sb[:], wO1[:])
        dg_sb = wpool.tile([128, 9 * 128], f16)
        nc.sync.dma_start(dg_sb[:], dg[:])

        u_tiles = [None] * NSTRIP

        def spectral(k):
            """proj_in + filter + inverse for strip k -> u_tiles[k]."""
            x_sb = xpool.tile([128, 3, NWB * 64], f16, tag="x")
            for kc in range(3):
                nc.sync.dma_start(x_sb[:, kc, :],
                                  xs[kc, :, k].rearrange("c wb f -> c (wb f)"))

            u_sb = upool.tile([128, 3, RSTRIP, WPAD], f16, tag="u")
            u_tiles[k] = u_sb
            nc.vector.memset(u_sb[:, :, :, 0:COL0], 0.0)
            nc.vector.memset(u_sb[:, :, :, COL0 + W:WPAD], 0.0)

            # halo strips only feed one u row into the dwconv
            r0, r1 = (7, 8) if k == 0 else (0, 1) if k == NSTRIP - 1 else (0, P)
            nr = r1 - r0
            for g in range(NG):
                zp = pZ.tile([128, 384], f32, tag="z", name=f"zp_{k}_{g}")
                for kc in range(3):
                    nc.tensor.matmul(zp[:],
                                     x_sb[:, kc, g * 128:(g + 1) * 128],
                                     wI_sb[:, kc, :],
                                     start=(kc == 0), stop=(kc == 2))
                zf = spool.tile([128, HID], f16, tag="zf")
                mode = ZF_PLAN[g % len(ZF_PLAN)]
                if no_pool_zf and mode == "b":
                    mode = "c"
                if no_zf_c and mode == "c":
                    mode = "a"
                if mode == "a":        # DVE multiply straight from PSUM
                    nc.vector.tensor_mul(zf[:], zp[:], d64_sb[:])
                else:                  # ACT copy, then SBUF multiply
                    zr = spool.tile([128, HID], f16, tag="zr")
                    nc.scalar.copy(zr[:], zp[:])
                    if mode == "b":
                        nc.gpsimd.tensor_tensor(zf[:], zr[:], d64_sb[:], MULT)
                    else:
                        nc.vector.tensor_mul(zf[:], zr[:], d64_sb[:])

                # inverse: ph layout [c, ch, p, r, s]
                ph = pH.tile([128, 3, 2, nr, 8], f32, tag="ph",
                             name=f"ph_{k}_{g}")
                for p in range(2):
                    for ch in range(3):
                        nc.tensor.matmul(
                            ph[:, ch, p],
                            zf[:, ch * 128:(ch + 1) * 128],
                            h64_sb[:, p, r0 * 8:r1 * 8])
                for p in range(2):
                    c0 = COL0 + g * 16 + 8 * p
                    nc.scalar.copy(u_sb[:, :, r0:r1, c0:c0 + 8], ph[:, :, p])

        def dw_sources(k, ch, dy, dx):
            """(out_rows_slice, src_ap) pairs for one tap over strip k's rows."""
            um, u0, up = u_tiles[k - 1], u_tiles[k], u_tiles[k + 1]
            off = COL0 + dx
            if dy == 0:
                return [((0, 8), u0[:, ch, :, off:off + W])]
            if dy == -1:
                return [((1, 8), u0[:, ch, 0:7, off:off + W]),
                        ((0, 1), um[:, ch, 7:8, off:off + W])]
            return [((0, 7), u0[:, ch, 1:8, off:off + W]),
                    ((7, 8), up[:, ch, 0:1, off:off + W])]

        def dwconv_gate_out(k):
            """dwconv + gelu gate + proj_out + store for own strip k (1..8)."""
            tp = lambda ch, dy, dx: taps_sb[:, ch, (dy + 1) * 3 + (dx + 1):
                                            (dy + 1) * 3 + (dx + 1) + 1]

            # ---- chunk 0: all taps on PE, gelu fused into eviction ----
            a0 = gpool.tile([128, RSTRIP, W], f16, tag="a0")
            for q in range(4):  # quarter-strips of 2 rows -> [128, 512] psum
                pv = pV.tile([128, 2, W], f32, tag="pv", name=f"pv_{k}_{q}")
                first = True
                for ti, (dy, dx) in enumerate(TAPS9):
                    for pi, ((r0, r1), src) in enumerate(dw_sources(k, 0, dy, dx)):
                        lo, hi = max(r0, 2 * q), min(r1, 2 * q + 2)
                        if lo >= hi:
                            continue
                        nc.tensor.matmul(
                            pv[:, lo - 2 * q:hi - 2 * q, :],
                            dg_sb[:, ti * 128:(ti + 1) * 128],
                            src[:, lo - r0:lo - r0 + (hi - lo)],
                            start=first,
                            stop=(ti == len(TAPS9) - 1),
                            skip_group_check=True)
                        first = False
                if no_gelu_fuse:
                    nc.scalar.copy(a0[:, 2 * q:2 * q + 2, :], pv[:])
                else:
                    nc.scalar.activation(a0[:, 2 * q:2 * q + 2, :], pv[:], GELU)

            # ---- chunks 1, 2: DVE/Pool/DMA mix ----
            v_t = [None, None]
            for ci, ch in enumerate((1, 2)):
                v = vpool.tile([128, RSTRIP, W], f16, tag=f"v{ch}")
                v_t[ci] = v
                # init with center tap (DVE tensor_scalar at 4x)
                nc.vector.tensor_scalar_mul(
                    v[:], u_tiles[k][:, ch, :, COL0:COL0 + W], tp(ch, 0, 0))
                for (dy, dx) in TAPS9:
                    eng = CH_PLAN[(dy, dx)]
                    if eng == "init":
                        continue
                    if eng == "dma" and no_dma_acc:
                        eng = "dve"
                    if eng == "act" and no_act_tap:
                        eng = "dve"
                    t = tpool.tile([128, RSTRIP, W], f16, tag="t")
                    for (r0, r1), src in dw_sources(k, ch, dy, dx):
                        if eng == "act":
                            nc.scalar.mul(t[:, r0:r1], src, tp(ch, dy, dx))
                        else:
                            nc.vector.tensor_scalar_mul(
                                t[:, r0:r1], src, tp(ch, dy, dx))
                    if eng == "dma":
                        nc.gpsimd.dma_start(
                            v.rearrange("c r w -> c (r w)"),
                            t.rearrange("c r w -> c (r w)"),
                            accum_op=ADD)
                    else:
                        nc.vector.tensor_tensor(
                            v.rearrange("c r w -> c (r w)"),
                            v.rearrange("c r w -> c (r w)"),
                            t.rearrange("c r w -> c (r w)"), ADD)
            v1, v2 = v_t

            # gated gelu
            g0 = gpool.tile([128, RSTRIP, W], f16, tag="g0")
            nc.vector.tensor_mul(g0[:], a0[:], v1[:])
            x2t = gpool.tile([64, RSTRIP, W], f16, tag="x2t")
            nc.gpsimd.dma_start(x2t[:], v2[64:128])
            at = gpool.tile([64, RSTRIP, W], f16, tag="at")
            nc.scalar.activation(at[:], v2[0:64], GELU)
            gt = gpool.tile([64, RSTRIP, W], f16, tag="gt")
            nc.vector.tensor_mul(gt[:], at[:], x2t[:])

            # proj_out
            o_sb = opool.tile([128, 3, RSTRIP * W], f16, tag="o")
            g0f = g0.rearrange("c r w -> c (r w)")
            gtf = gt.rearrange("c r w -> c (r w)")
            NT = RSTRIP * W // 512
            for m in range(3):
                for nt in range(NT):
                    pso = pA.tile([128, 512], f32, tag="ps",
                                  name=f"pso_{k}_{m}_{nt}")
                    nc.tensor.matmul(pso[:], wO0_sb[:, m * 128:(m + 1) * 128],
                                     g0f[:, nt * 512:(nt + 1) * 512],
                                     start=True, stop=False)
                    nc.tensor.matmul(pso[:], wO1_sb[:, m * 128:(m + 1) * 128],
                                     gtf[:, nt * 512:(nt + 1) * 512],
                                     start=False, stop=True)
                    dst = o_sb[:, m, nt * 512:(nt + 1) * 512]
                    if (m * NT + nt) % 6 == 5 and not no_dve_pso:
                        nc.vector.tensor_copy(dst, pso[:])
                    else:
                        nc.scalar.copy(dst, pso[:])
            nc.gpsimd.dma_start(
                y[:, (k - 1) * RSTRIP:k * RSTRIP, :]
                .rearrange("(m c) r w -> c m r w", m=3),
                o_sb.rearrange("c m (r w) -> c m r w", r=RSTRIP))

        spec_only = os.environ.get("K_SPEC_ONLY") == "1"
        for k in range(NSTRIP):
            spectral(k)
            if spec_only:
                if 1 <= k <= 8:
                    nc.gpsimd.dma_start(
                        y[0:128, (k - 1) * RSTRIP:k * RSTRIP, :],
                        u_tiles[k][:, 0, :, COL0:COL0 + W])
            elif k >= 2:
                dwconv_gate_out(k - 1)

    nc.compile()
    return nc


def _prepare_weights(fft_filter, w_in, w_dw, w_out):
    V, L = _build_spectral64()
    perm = _PERM
    d64 = (fft_filter.reshape(HID, 40) @ L.T)[perm].T      # [64, hid']
    d64p = np.ascontiguousarray(np.concatenate([d64, d64], axis=0)) \
        .astype(np.float16)                                # [128, hid']
    wI = np.ascontiguousarray(w_in[perm, :].T).astype(np.float16)
    h64 = np.zeros((128, 2, 64), np.float16)               # zero-padded to
    h64[0:64, 0] = V.T.astype(np.float16)                  # full-tile K=128
    h64[64:128, 1] = V.T.astype(np.float16)
    tapsP = w_dw[perm, 0].reshape(3, 128, 9).transpose(1, 0, 2)
    tapsP = np.ascontiguousarray(tapsP).astype(np.float32)
    wO0 = np.ascontiguousarray(w_out[:, 0:128].T).astype(np.float16)
    wO1 = np.ascontiguousarray(w_out[:, 128:192].T).astype(np.float16)
    dgm = np.zeros((128, 9 * 128), np.float16)
    for i in range(9):
        np.fill_diagonal(dgm[:, i * 128:(i + 1) * 128], tapsP[:, 0, i])
    return dict(wI=wI, h64=h64, d64=d64p, taps=tapsP,
                wO0=wO0, wO1=wO1, dg=dgm)


def build_in_maps(inputs):
    wd = _prepare_weights(np.asarray(inputs["fft_filter"]),
                          np.asarray(inputs["w_in"]),
                          np.asarray(inputs["w_dw"]), np.asarray(inputs["w_out"]))
    V, _ = _build_spectral64()
    x16 = np.asarray(inputs["x"]).astype(np.float16).astype(np.float32)
    HB = H // P
    xp = x16.reshape(B, HID, HB, P, NWB, P).transpose(0, 1, 2, 4, 3, 5) \
        .reshape(B, HID, HB, NWB, 64)
    xz = (xp.reshape(-1, 64) @ V.astype(np.float32)) \
        .reshape(B, HID, HB, NWB, 64).astype(np.float16)
    perm = _PERM
    in_maps = []
    for core in range(NCORES):
        b, j = divmod(core, 4)
        lo, hi = 8 * j - 1, 8 * j + 9        # patch-strip indices incl halo
        clo, chi = max(lo, 0), min(hi, HB)
        slab = np.zeros((3, 128, NSTRIP, NWB, 64), np.float16)
        for kc in range(3):
            slab[kc, :, clo - lo:clo - lo + (chi - clo)] = \
                xz[b, kc * 128:(kc + 1) * 128, clo:chi]
        in_maps.append({"xs": np.ascontiguousarray(slab), **wd})
    return in_maps


def kernel(x, fft_filter, w_in, w_dw, w_out):
    global _compiled
    import os
    os.environ["BASS_NEVER_TRACE"] = "1"
    from concourse.bass_utils import run_bass_kernel_spmd

    if _compiled is None:
        _compiled = _build_program()
    nc = _compiled

    in_maps = build_in_maps(dict(x=x, fft_filter=fft_filter, w_in=w_in,
                                 w_dw=w_dw, w_out=w_out))

    res = run_bass_kernel_spmd(nc, in_maps, list(range(NCORES)))
    global last_results
    last_results = res
    out = np.empty((B, DIM, H, W), np.float32)
    for core in range(NCORES):
        b, j = divmod(core, 4)
        out[b, :, 64 * j:64 * j + OWN_R, :] = \
            res.results[core]["y"].astype(np.float32)
    return out
